# revision 1
# baseline (speedup 1.0000x reference)
"""GAT-Transformer (2-layer) distributed Bass kernel for 8 Trainium2 NeuronCores.

Sharding: nodes partitioned across 8 cores (5000/core, padded to 5120 = 40
blocks x 128). Edges partitioned by destination node and sorted by dst, so
segment-softmax and scatter-aggregate are device-local. Per layer, each core
computes h = LN(x) @ W plus attention logit vectors for its node shard, then
an AllGather shares the concatenated [h | alpha_src] rows (bf16) with every
core. The edge phase gathers source rows with indirect DMA (128 edges/call),
builds a one-hot scatter matrix S per 128-edge subtile with iota+is_equal,
and uses TensorE matmuls S.T @ [msg|ex] accumulated in PSUM for both the
softmax denominators and the message aggregation. Softmax max-subtraction is
skipped: alpha = leaky_relu(...) of LN-normalized quantities is bounded (|a|
of order 1), so exp() cannot overflow and the result matches the reference
to well below fp32 noise.

LayerNorm gains/biases are folded into the adjacent weight matrices on
device (W' = diag(g) @ W once per layer; the bias becomes a rank-1 matmul
accumulated into the same PSUM group), so the per-block LN is just
mean/var/rsqrt + one fused (x-mu)*rstd op. The FF first matmul is computed
transposed (lhsT = W1 column chunks, rhs = LN(x)^T) so no activation
transposes are needed there; gelu bias rides the ACT per-partition bias port.

Host-side preprocessing only reorders/partitions data (edge sort by dst,
index remapping, weight layout transposes) - all FLOPs on tensor data happen
on device.
"""
import math

import numpy as np
import ml_dtypes

import concourse.bacc as bacc
import concourse.mybir as mybir
import concourse.tile as tile
from concourse.bass import IndirectOffsetOnAxis
from concourse.masks import make_identity

# model dims (fixed by the problem)
D = 128         # model dim
H = 4           # heads
C = 128         # per-head channels
HC = H * C      # 512
MLP = 512
DEPTH = 2
NEG_SLOPE = 0.2
EPS_SM = 1e-16
EPS_LN = 1e-5

CORES = 8
P = 128

# scheduling knobs (tuned via TimelineSim + device A/B)
TUNE = {"sb": 8, "hg": 8, "pgat": 1, "ptr": 2, "pmm": 3, "palp": 1, "pdn": 1}
F = HC + 8      # h row (512) + alpha_src (4) + alpha_dst (4)

BF16 = mybir.dt.bfloat16
F32 = mybir.dt.float32
I32 = mybir.dt.int32
AF = mybir.ActivationFunctionType
ALU = mybir.AluOpType
AX = mybir.AxisListType


# ----------------------------------------------------------------------------
# device kernel builder
# ----------------------------------------------------------------------------

def build_nc(npad, ksub, rep=1, do_edges=True, do_gather=True, do_dense=True,
             do_adst=True, do_msg=True, do_coll=True):
    """Build the Bass program for one core. npad: padded local node count
    (multiple of 128); ksub: 128-edge subtiles per 128-node block.
    rep>1 repeats the whole network body (timing only). do_* flags disable
    pipeline stages for perf bisection (results wrong when False)."""
    nblk = npad // P
    nsub = nblk * ksub
    ng = CORES * npad  # global padded nodes

    from contextlib import ExitStack
    nc = bacc.Bacc(None, target_bir_lowering=False, debug=False)
    with tile.TileContext(nc) as tc, ExitStack() as es:
        dram = es.enter_context(tc.tile_pool(name="dram", bufs=1, space="DRAM"))
        const = es.enter_context(tc.tile_pool(name="const", bufs=1))
        wpool = es.enter_context(tc.tile_pool(name="wpool", bufs=1))
        sb = es.enter_context(tc.tile_pool(name="sb", bufs=TUNE["sb"]))
        hg = es.enter_context(tc.tile_pool(name="hg", bufs=TUNE["hg"]))
        pgat = es.enter_context(tc.tile_pool(name="pgat", bufs=TUNE["pgat"], space="PSUM"))
        palpha = es.enter_context(tc.tile_pool(name="palpha", bufs=TUNE["palp"], space="PSUM"))
        pdn = es.enter_context(tc.tile_pool(name="pdn", bufs=TUNE["pdn"], space="PSUM"))
        ptr = es.enter_context(tc.tile_pool(name="ptr", bufs=TUNE["ptr"], space="PSUM"))
        pmm = es.enter_context(tc.tile_pool(name="pmm", bufs=TUNE["pmm"], space="PSUM"))
        p512 = pmm
        p128 = pmm

        # ---- I/O ------------------------------------------------------------
        def einp(name, shape, dtype=F32):
            return dram.tile(shape, dtype, kind="ExternalInput", name=name,
                             uniquify=False)

        x_in = einp("x", [npad, D])
        src_idx = einp("src_idx", [P, nsub], I32)      # global padded row in h_full
        dst_loc = einp("dst_loc", [P, nsub])           # dst within block, 255=pad
        ea_in = einp("ea", [P, nsub])                  # edge_attr, sorted order
        w_gat = einp("gat_W", [DEPTH, D, HC])
        a_srcT = einp("att_srcT", [DEPTH, C, H])
        a_dstT = einp("att_dstT", [DEPTH, C, H])
        ew_T = einp("edge_WT", [DEPTH, C, H])
        ae_T = einp("att_edgeT", [DEPTH, C, H])
        gb_T = einp("gat_biasT", [DEPTH, C, H])
        w_qf = einp("qf_W", [DEPTH, HC, D])
        b_qf = einp("qf_b", [DEPTH, D])
        ln1gT = einp("ln1_gT", [DEPTH, D, 1])
        ln1bT = einp("ln1_bT", [DEPTH, D, 1])
        ln2gT = einp("ln2_gT", [DEPTH, D, 1])
        ln2bT = einp("ln2_bT", [DEPTH, D, 1])
        w_f1 = einp("ff_W1", [DEPTH, D, MLP])
        b_f1T = einp("ff_b1T", [DEPTH, P, MLP // P])
        w_f2 = einp("ff_W2", [DEPTH, MLP, D])
        b_f2 = einp("ff_b2", [DEPTH, D])

        x_out = dram.tile([npad, D], F32, kind="ExternalOutput", name="x_out",
                          uniquify=False)

        h_locs = [dram.tile([npad, F], BF16, name=f"h_loc{d}")
                  for d in range(DEPTH * rep)]
        h_fulls = [dram.tile([ng, F], BF16,
                             addr_space="Shared" if do_coll else "Local",
                             name=f"h_full{d}")
                   for d in range(DEPTH * rep)]

        # ---- static constants ----------------------------------------------
        iota_i = const.tile([P, P], I32)
        nc.gpsimd.iota(iota_i[:], pattern=[[1, P]], base=0, channel_multiplier=0)
        iota_bf = const.tile([P, P], BF16)
        nc.vector.tensor_copy(out=iota_bf[:], in_=iota_i[:])
        ident_bf = const.tile([P, P], BF16)
        make_identity(nc, ident_bf)
        ones1_bf = const.tile([1, P], BF16)
        nc.vector.memset(ones1_bf[:], 1.0)
        one11_bf = const.tile([1, 1], BF16)
        nc.vector.memset(one11_bf[:], 1.0)
        epsln = const.tile([P, 1], F32)
        nc.vector.memset(epsln[:], EPS_LN)

        x_sb = const.tile([P, nblk, D], F32)
        nc.sync.dma_start(out=x_sb[:], in_=x_in[:].rearrange("(b p) f -> p b f", p=P))
        srcx_sb = const.tile([P, nsub], I32)
        nc.sync.dma_start(out=srcx_sb[:], in_=src_idx[:])
        dstl_bf = const.tile([P, nsub], BF16)
        nc.gpsimd.dma_start(out=dstl_bf[:], in_=dst_loc[:])
        ea_sb = const.tile([P, nsub], F32)
        nc.sync.dma_start(out=ea_sb[:], in_=ea_in[:])
        adst_all = const.tile([P, nblk, H], BF16)

        for rd in range(DEPTH * rep):
            d = rd % DEPTH
            h_loc = h_locs[rd]
            h_full = h_fulls[rd]
            # ---------------- per-layer setup -------------------------------
            w_bf = wpool.tile([D, HC], BF16, name="w_bf")
            nc.gpsimd.dma_start(out=w_bf[:], in_=w_gat[d])   # cast f32->bf16
            qfw_bf = wpool.tile([P, H, D], BF16, name="qfw_bf")
            nc.gpsimd.dma_start(out=qfw_bf[:],
                                in_=w_qf[d].rearrange("(c p) n -> p c n", p=P))
            f1w_bf = wpool.tile([D, MLP], BF16, name="f1w_bf")
            nc.gpsimd.dma_start(out=f1w_bf[:], in_=w_f1[d])
            f2w_bf = wpool.tile([P, H, D], BF16, name="f2w_bf")
            nc.gpsimd.dma_start(out=f2w_bf[:],
                                in_=w_f2[d].rearrange("(c p) n -> p c n", p=P))
            att_bf = wpool.tile([C, 4, H], BF16, name="att_bf")  # src,dst,ew,ae cols
            nc.gpsimd.dma_start(out=att_bf[:, 0, :], in_=a_srcT[d])
            nc.gpsimd.dma_start(out=att_bf[:, 1, :], in_=a_dstT[d])
            nc.gpsimd.dma_start(out=att_bf[:, 2, :], in_=ew_T[d])
            nc.gpsimd.dma_start(out=att_bf[:, 3, :], in_=ae_T[d])
            gbT_bf = wpool.tile([C, H], BF16, name="gbT_bf")
            nc.gpsimd.dma_start(out=gbT_bf[:], in_=gb_T[d])
            qfb_bf = wpool.tile([1, D], BF16, name="qfb_bf")
            nc.gpsimd.dma_start(out=qfb_bf[:], in_=b_qf[d:d + 1, :])
            f2b_bf = wpool.tile([1, D], BF16, name="f2b_bf")
            nc.gpsimd.dma_start(out=f2b_bf[:], in_=b_f2[d:d + 1, :])
            f1bT = wpool.tile([P, MLP // P], F32, name="f1bT")
            nc.sync.dma_start(out=f1bT[:], in_=b_f1T[d])
            g1c = wpool.tile([D, 1], F32, name="g1c")
            nc.sync.dma_start(out=g1c[:], in_=ln1gT[d])
            b1c = wpool.tile([D, 1], BF16, name="b1c")
            nc.gpsimd.dma_start(out=b1c[:], in_=ln1bT[d])
            g2c = wpool.tile([D, 1], F32, name="g2c")
            nc.sync.dma_start(out=g2c[:], in_=ln2gT[d])
            b2c = wpool.tile([D, 1], BF16, name="b2c")
            nc.gpsimd.dma_start(out=b2c[:], in_=ln2bT[d])

            # W transposed per head (for Wa = W_h @ a_h)
            wT_bf = wpool.tile([C, H, D], BF16, name="wT_bf")
            for h in range(H):
                pst = ptr.tile([P, P], BF16, space="PSUM", name="pst")
                nc.tensor.transpose(out=pst[:], in_=w_bf[:, h * C:(h + 1) * C],
                                    identity=ident_bf[:])
                nc.any.tensor_copy(out=wT_bf[:, h, :], in_=pst[:])
            # Wa[:, 0:4] = W_h @ a_src_h ; [:, 4:8] = W_h @ a_dst_h
            pwa = p128.tile([D, 2 * H], F32, space="PSUM", name="p128t", tag="pmmt")
            for h in range(H):
                nc.tensor.matmul(out=pwa[:, h:h + 1], lhsT=wT_bf[:, h, :],
                                 rhs=att_bf[:, 0, h:h + 1], start=True, stop=True)
                nc.tensor.matmul(out=pwa[:, H + h:H + h + 1], lhsT=wT_bf[:, h, :],
                                 rhs=att_bf[:, 1, h:h + 1], start=True, stop=True)
            wa_bf = wpool.tile([D, 2 * H], BF16, name="wa_bf")
            nc.any.tensor_copy(out=wa_bf[:], in_=pwa[:])
            # fold LN1 gain into W / Wa; LN1 bias becomes rank-1 rows
            w_s = wpool.tile([D, HC], BF16, name="w_s")
            nc.vector.tensor_scalar_mul(out=w_s[:], in0=w_bf[:], scalar1=g1c[:, :1])
            wa_s = wpool.tile([D, 2 * H], BF16, name="wa_s")
            nc.vector.tensor_scalar_mul(out=wa_s[:], in0=wa_bf[:], scalar1=g1c[:, :1])
            pbw = p512.tile([1, HC], F32, space="PSUM", name="pbw", tag="pmmt")
            nc.tensor.matmul(out=pbw[:], lhsT=b1c[:], rhs=w_bf[:],
                             start=True, stop=True)
            bw_row = wpool.tile([1, HC], BF16, name="bw_row")
            nc.any.tensor_copy(out=bw_row[:], in_=pbw[:])
            pbwa = p128.tile([1, 2 * H], F32, space="PSUM", name="pbwa", tag="pmmt")
            nc.tensor.matmul(out=pbwa[:], lhsT=b1c[:], rhs=wa_bf[:],
                             start=True, stop=True)
            bwa_row = wpool.tile([1, 2 * H], BF16, name="bwa_row")
            nc.any.tensor_copy(out=bwa_row[:], in_=pbwa[:])
            # fold LN2 gain into ff_W1; LN2 bias -> rank-1 row
            f1w_s = wpool.tile([D, MLP], BF16, name="f1w_s")
            nc.vector.tensor_scalar_mul(out=f1w_s[:], in0=f1w_bf[:], scalar1=g2c[:, :1])
            pbw1 = p512.tile([1, MLP], F32, space="PSUM", name="pbw1", tag="pmmt")
            nc.tensor.matmul(out=pbw1[:], lhsT=b2c[:], rhs=f1w_bf[:],
                             start=True, stop=True)
            bw1_row = wpool.tile([1, MLP], BF16, name="bw1_row")
            nc.any.tensor_copy(out=bw1_row[:], in_=pbw1[:])

            # we_dot[h] = <edge_W_h, a_edge_h>, replicated [P, ksub*H]
            pwe = p128.tile([1, H], F32, space="PSUM", name="p128t2", tag="pmmt")
            for h in range(H):
                nc.tensor.matmul(out=pwe[:, h:h + 1], lhsT=att_bf[:, 2, h:h + 1],
                                 rhs=att_bf[:, 3, h:h + 1], start=True, stop=True)
            wd_row = wpool.tile([1, H], BF16, name="wd_row")
            nc.any.tensor_copy(out=wd_row[:], in_=pwe[:])
            wd_rep = wpool.tile([1, ksub, H], BF16, name="wd_rep")
            nc.vector.tensor_copy(
                out=wd_rep[:],
                in_=wd_row[:].unsqueeze(1).to_broadcast([1, ksub, H]))
            pwr = p512.tile([P, ksub * H], F32, space="PSUM", name="p512t", tag="pmmt")
            nc.tensor.matmul(out=pwr[:], lhsT=ones1_bf[:],
                             rhs=wd_rep[:].rearrange("a k h -> a (k h)"),
                             start=True, stop=True)
            wedot = wpool.tile([P, ksub * H], F32, name="wedot")
            nc.any.tensor_copy(out=wedot[:], in_=pwr[:])

            # qfb_eff = gat_bias @ qf_W + qf_b   -> [1, D]
            pqb = p128.tile([1, D], F32, space="PSUM", name="p128t3", tag="pmmt")
            for cch in range(H):
                nc.tensor.matmul(out=pqb[:], lhsT=gbT_bf[:, cch:cch + 1],
                                 rhs=qfw_bf[:, cch, :], start=(cch == 0), stop=False)
            nc.tensor.matmul(out=pqb[:], lhsT=one11_bf[:], rhs=qfb_bf[:],
                             start=False, stop=True)
            qfbe = wpool.tile([1, D], BF16, name="qfbe")
            nc.any.tensor_copy(out=qfbe[:], in_=pqb[:])

            # ---------------- layer norm helper -----------------------------
            def layer_norm_t(x_blk, name):
                mu = sb.tile([P, 1], F32, name=f"mu{name}")
                nc.vector.tensor_reduce(out=mu[:], in_=x_blk, axis=AX.X, op=ALU.add)
                nc.vector.tensor_scalar_mul(out=mu[:], in0=mu[:], scalar1=1.0 / D)
                xc = sb.tile([P, D], F32, name=f"xc{name}")
                nc.vector.tensor_scalar(out=xc[:], in0=x_blk, scalar1=mu[:, :1],
                                        scalar2=None, op0=ALU.subtract)
                sq = sb.tile([P, D], F32, name=f"sq{name}")
                nc.scalar.activation(out=sq[:], in_=xc[:], func=AF.Square)
                var = sb.tile([P, 1], F32, name=f"var{name}")
                nc.vector.tensor_reduce(out=var[:], in_=sq[:], axis=AX.X, op=ALU.add)
                std = sb.tile([P, 1], F32, name=f"std{name}")
                nc.scalar.activation(out=std[:], in_=var[:], func=AF.Sqrt,
                                     bias=epsln[:, :1], scale=1.0 / D)
                rstd = sb.tile([P, 1], F32, name=f"rstd{name}")
                nc.vector.reciprocal(out=rstd[:], in_=std[:])
                xn = sb.tile([P, D], BF16, name=f"xn{name}")
                nc.vector.tensor_scalar_mul(out=xn[:], in0=xc[:], scalar1=rstd[:, :1])
                # transpose -> [D, P] bf16
                pst = ptr.tile([P, P], BF16, space="PSUM", name="pst")
                nc.tensor.transpose(out=pst[:], in_=xn[:], identity=ident_bf[:])
                xnT = sb.tile([P, P], BF16, name=f"xnT{name}")
                nc.any.tensor_copy(out=xnT[:], in_=pst[:])
                return xnT

            # ---------------- stage A: h production -------------------------
            for b in range(nblk):
                xnT = layer_norm_t(x_sb[:, b, :], "A")
                ph = p512.tile([P, HC], F32, space="PSUM", name="ph", tag="pmmt")
                nc.tensor.matmul(out=ph[:], lhsT=xnT[:], rhs=w_s[:],
                                 start=True, stop=False)
                nc.tensor.matmul(out=ph[:], lhsT=ones1_bf[:], rhs=bw_row[:],
                                 start=False, stop=True)
                pa8 = p128.tile([P, 2 * H], F32, space="PSUM", name="pa8", tag="pmmt")
                nc.tensor.matmul(out=pa8[:], lhsT=xnT[:], rhs=wa_s[:],
                                 start=True, stop=False)
                nc.tensor.matmul(out=pa8[:], lhsT=ones1_bf[:], rhs=bwa_row[:],
                                 start=False, stop=True)
                h_sb = sb.tile([P, F], BF16, name="h_sb")
                nc.scalar.activation(out=h_sb[:, 0:HC], in_=ph[:], func=AF.Copy)
                nc.any.tensor_copy(out=h_sb[:, HC:HC + 2 * H], in_=pa8[:])
                nc.any.tensor_copy(out=adst_all[:, b, :], in_=pa8[:, H:2 * H])
                nc.sync.dma_start(out=h_loc[b * P:(b + 1) * P, :], in_=h_sb[:])

            # ---------------- stage B: all-gather ---------------------------
            if do_coll:
                nc.gpsimd.collective_compute(
                    "AllGather", ALU.bypass,
                    replica_groups=[list(range(CORES))],
                    ins=[h_loc[:].opt()],
                    outs=[h_full[:].opt()],
                )
            else:
                nc.sync.dma_start(out=h_full[0:npad, :], in_=h_loc[:])

            # ---------------- tail: qf + ff for one block --------------------
            def emit_tail(b, g_sb):
                if not do_dense:
                    return
                # qf: x += g @ qf_W + qfb_eff
                px = p128.tile([P, D], F32, space="PSUM", name="px", tag="pmmt")
                for cch in range(H):
                    pst = ptr.tile([P, P], BF16, space="PSUM", name="pst")
                    nc.tensor.transpose(out=pst[:], in_=g_sb[:, cch * P:(cch + 1) * P],
                                        identity=ident_bf[:])
                    gT = sb.tile([P, P], BF16, name="gT")
                    nc.any.tensor_copy(out=gT[:], in_=pst[:])
                    nc.tensor.matmul(out=px[:], lhsT=gT[:], rhs=qfw_bf[:, cch, :],
                                     start=(cch == 0), stop=False)
                nc.tensor.matmul(out=px[:], lhsT=ones1_bf[:], rhs=qfbe[:],
                                 start=False, stop=True)
                nc.vector.tensor_add(out=x_sb[:, b, :], in0=x_sb[:, b, :], in1=px[:])

                # ff: x += gelu(LN2(x) @ W1 + b1) @ W2 + b2
                xn2T = layer_norm_t(x_sb[:, b, :], "B")
                pa1 = p512.tile([P, MLP], F32, space="PSUM", name="pa1", tag="pmmt")
                a1T = sb.tile([P, H, P], BF16, name="a1T")
                for cch in range(MLP // P):
                    nc.tensor.matmul(out=pa1[:, cch * P:(cch + 1) * P],
                                     lhsT=f1w_s[:, cch * P:(cch + 1) * P],
                                     rhs=xn2T[:], start=True, stop=False)
                    nc.tensor.matmul(out=pa1[:, cch * P:(cch + 1) * P],
                                     lhsT=bw1_row[:, cch * P:(cch + 1) * P],
                                     rhs=ones1_bf[:], start=False, stop=True)
                    nc.scalar.activation(out=a1T[:, cch, :],
                                         in_=pa1[:, cch * P:(cch + 1) * P],
                                         func=AF.Gelu_apprx_tanh,
                                         bias=f1bT[:, cch:cch + 1])
                pf2 = p128.tile([P, D], F32, space="PSUM", name="pf2", tag="pmmt")
                for cch in range(MLP // P):
                    nc.tensor.matmul(out=pf2[:], lhsT=a1T[:, cch, :],
                                     rhs=f2w_bf[:, cch, :],
                                     start=(cch == 0), stop=False)
                nc.tensor.matmul(out=pf2[:], lhsT=ones1_bf[:], rhs=f2b_bf[:],
                                 start=False, stop=True)
                nc.vector.tensor_add(out=x_sb[:, b, :], in0=x_sb[:, b, :], in1=pf2[:])

            # ---------------- stage C: edge phase + rest of layer -----------
            for b in range(nblk):
                if not do_edges:
                    g_sb = sb.tile([P, HC], BF16, name="g_sb")
                    nc.vector.memset(g_sb[:], 0.0)
                    emit_tail(b, g_sb)
                    continue
                hgat = hg.tile([P, ksub, F], BF16, name="hgat")
                palp = palpha.tile([P, ksub * H], F32, space="PSUM", name="palp",
                                   tag="palp")
                s_all = sb.tile([P, ksub, P], BF16, name="s_all", bufs=2)
                nc.vector.tensor_tensor(
                    out=s_all[:],
                    in0=iota_bf[:].unsqueeze(1).to_broadcast([P, ksub, P]),
                    in1=dstl_bf[:, b * ksub:(b + 1) * ksub].unsqueeze(2)
                        .to_broadcast([P, ksub, P]),
                    op=ALU.is_equal)
                s_tiles = [s_all[:, k, :] for k in range(ksub)]
                for k in range(ksub):
                    s = b * ksub + k
                    if do_gather:
                        nc.gpsimd.indirect_dma_start(
                            out=hgat[:, k, :], out_offset=None, in_=h_full[:],
                            in_offset=IndirectOffsetOnAxis(ap=srcx_sb[:, s:s + 1],
                                                           axis=0))
                    else:
                        nc.sync.dma_start(out=hgat[:, k, :],
                                          in_=h_full[b * P:(b + 1) * P, :])
                    s_bf = s_tiles[k]
                    if do_adst:
                        pst = ptr.tile([P, P], BF16, space="PSUM", name="pst")
                        nc.tensor.transpose(out=pst[:], in_=s_bf,
                                            identity=ident_bf[:])
                        sT_bf = sb.tile([P, P], BF16, name="sT_bf")
                        if k % 2 == 0:
                            nc.vector.tensor_copy(out=sT_bf[:], in_=pst[:])
                        else:
                            nc.scalar.activation(out=sT_bf[:], in_=pst[:],
                                                 func=AF.Copy)
                        nc.tensor.matmul(out=palp[:, k * H:(k + 1) * H],
                                         lhsT=sT_bf[:], rhs=adst_all[:, b, :],
                                         start=True, stop=True)
                # alpha = asrc + adst + ea*wedot; lrelu; exp
                # (two half-block batches so scatter work can start earlier)
                kh = (ksub + 1) // 2
                halves = [(0, kh), (kh, ksub)] if ksub > kh else [(0, ksub)]
                ex_f = sb.tile([P, ksub * H], F32, name="ex_f")
                ex_bf = sb.tile([P, ksub * H], BF16, name="ex_bf")
                for (k0, k1) in halves:
                    kk = k1 - k0
                    if kk <= 0:
                        continue
                    csl = slice(k0 * H, k1 * H)
                    al1 = sb.tile([P, kh * H], F32, name="al1")
                    a1v = al1[:, :kk * H]
                    if do_adst:
                        nc.vector.tensor_tensor(
                            out=a1v.rearrange("p (k h) -> p k h", h=H),
                            in0=palp[:, csl].rearrange("p (k h) -> p k h", h=H),
                            in1=hgat[:, k0:k1, HC:HC + H], op=ALU.add)
                    else:
                        nc.vector.tensor_copy(
                            out=a1v.rearrange("p (k h) -> p k h", h=H),
                            in_=hgat[:, k0:k1, HC:HC + H])
                    aef = sb.tile([P, kh, H], F32, name="aef")
                    nc.vector.tensor_tensor(
                        out=aef[:, :kk, :],
                        in0=wedot[:, csl].rearrange("p (k h) -> p k h", h=H),
                        in1=ea_sb[:, b * ksub + k0:b * ksub + k1].unsqueeze(2)
                            .to_broadcast([P, kk, H]),
                        op=ALU.mult)
                    al2 = sb.tile([P, kh * H], F32, name="al2")
                    nc.vector.tensor_tensor(
                        out=al2[:, :kk * H], in0=a1v,
                        in1=aef[:, :kk, :].rearrange("p k h -> p (k h)"),
                        op=ALU.add)
                    lr = sb.tile([P, kh * H], F32, name="lr")
                    nc.vector.scalar_tensor_tensor(
                        out=lr[:, :kk * H], in0=al2[:, :kk * H],
                        scalar=NEG_SLOPE, in1=al2[:, :kk * H],
                        op0=ALU.mult, op1=ALU.max)
                    nc.scalar.activation(out=ex_f[:, csl], in_=lr[:, :kk * H],
                                         func=AF.Exp)
                    nc.vector.tensor_copy(out=ex_bf[:, csl], in_=ex_f[:, csl])

                pg = pgat.tile([P, HC], F32, space="PSUM", name="pg")
                pd = pdn.tile([P, H], F32, space="PSUM", name="pd")
                for k in range(ksub):
                    if do_msg:
                        msg = sb.tile([P, HC], BF16, name="msg")
                        for h in range(H):
                            nc.vector.tensor_scalar_mul(
                                out=msg[:, h * C:(h + 1) * C],
                                in0=hgat[:, k, h * C:(h + 1) * C],
                                scalar1=ex_f[:, k * H + h:k * H + h + 1])
                        rhs_msg = msg[:]
                    else:
                        rhs_msg = hgat[:, k, 0:HC]
                    nc.tensor.matmul(out=pg[:], lhsT=s_tiles[k], rhs=rhs_msg,
                                     start=(k == 0), stop=(k == ksub - 1))
                    nc.tensor.matmul(out=pd[:], lhsT=s_tiles[k],
                                     rhs=ex_bf[:, k * H:(k + 1) * H],
                                     start=(k == 0), stop=(k == ksub - 1))
                # normalize: g = unnorm * (1/(den+eps)) per head
                den = sb.tile([P, H], F32, name="den")
                nc.vector.tensor_scalar_add(out=den[:], in0=pd[:], scalar1=EPS_SM)
                rec = sb.tile([P, H], F32, name="rec")
                nc.vector.reciprocal(out=rec[:], in_=den[:])
                g_sb = sb.tile([P, HC], BF16, name="g_sb")
                for h in range(H):
                    nc.vector.tensor_scalar_mul(out=g_sb[:, h * C:(h + 1) * C],
                                                in0=pg[:, h * C:(h + 1) * C],
                                                scalar1=rec[:, h:h + 1])
                emit_tail(b, g_sb)

        nc.sync.dma_start(out=x_out[:].rearrange("(b p) f -> p b f", p=P),
                          in_=x_sb[:])
    nc.finalize()
    return nc


# ----------------------------------------------------------------------------
# host-side sharding / preprocessing
# ----------------------------------------------------------------------------

def preprocess(x, edge_index, edge_attr):
    n = x.shape[0]
    e = edge_index.shape[1]
    assert n % CORES == 0
    nloc = n // CORES
    npad = ((nloc + P - 1) // P) * P
    nblk = npad // P

    src = np.asarray(edge_index[0], dtype=np.int64)
    dst = np.asarray(edge_index[1], dtype=np.int64)
    # remap to padded ids
    src_p = (src // nloc) * npad + (src % nloc)
    dst_p = (dst // nloc) * npad + (dst % nloc)
    dev = dst // nloc

    ea = np.asarray(edge_attr, dtype=np.float32).reshape(-1)

    per_dev = []
    ksub = 1
    for dcore in range(CORES):
        sel = np.nonzero(dev == dcore)[0]
        order = np.argsort(dst_p[sel], kind="stable")
        sel = sel[order]
        dloc = dst_p[sel] - dcore * npad          # [0, npad)
        blk = dloc // P
        cnt = np.bincount(blk, minlength=nblk)
        ksub = max(ksub, int(math.ceil(cnt.max() / P)) if len(sel) else 1)
        per_dev.append((sel, dloc, blk, cnt))

    nsub = nblk * ksub
    cap = ksub * P
    in_edge = []
    for dcore in range(CORES):
        sel, dloc, blk, cnt = per_dev[dcore]
        srcx = np.zeros((nblk, cap), dtype=np.int32)
        dl = np.full((nblk, cap), 255.0, dtype=np.float32)
        eav = np.zeros((nblk, cap), dtype=np.float32)
        starts = np.concatenate([[0], np.cumsum(cnt)])
        for b in range(nblk):
            s0, s1 = starts[b], starts[b + 1]
            m = s1 - s0
            srcx[b, :m] = src_p[sel[s0:s1]]
            dl[b, :m] = dloc[s0:s1] - b * P
            eav[b, :m] = ea[sel[s0:s1]]
        # [nblk, cap] -> [P, nsub]: subtile k of block b at col b*ksub+k,
        # edge slot p on partition p
        def to_tiles(a):
            return np.ascontiguousarray(
                a.reshape(nblk, ksub, P).transpose(2, 0, 1).reshape(P, nsub))
        in_edge.append({
            "src_idx": to_tiles(srcx),
            "dst_loc": to_tiles(dl),
            "ea": to_tiles(eav),
        })
    return nloc, npad, ksub, in_edge


def make_in_maps(inputs):
    x = np.asarray(inputs["x"], dtype=np.float32)
    nloc, npad, ksub, in_edge = preprocess(x, inputs["edge_index"],
                                           inputs["edge_attr"])

    def f32(name):
        return np.asarray(inputs[name], dtype=np.float32)

    w_gat = f32("gat_W")
    att_srcT = np.ascontiguousarray(f32("att_src").transpose(0, 2, 1))
    att_dstT = np.ascontiguousarray(f32("att_dst").transpose(0, 2, 1))
    edge_WT = np.ascontiguousarray(
        f32("edge_W").reshape(DEPTH, H, C).transpose(0, 2, 1))
    att_edgeT = np.ascontiguousarray(f32("att_edge").transpose(0, 2, 1))
    gat_biasT = np.ascontiguousarray(
        f32("gat_bias").reshape(DEPTH, H, C).transpose(0, 2, 1))
    ff_b1T = np.ascontiguousarray(
        f32("ff_b1").reshape(DEPTH, MLP // P, P).transpose(0, 2, 1))


    shared = {
        "gat_W": w_gat,
        "att_srcT": att_srcT, "att_dstT": att_dstT,
        "edge_WT": edge_WT, "att_edgeT": att_edgeT, "gat_biasT": gat_biasT,
        "qf_W": f32("qf_W"), "qf_b": f32("qf_b"),
        "ln1_gT": f32("ln1_g")[:, :, None], "ln1_bT": f32("ln1_b")[:, :, None],
        "ln2_gT": f32("ln2_g")[:, :, None], "ln2_bT": f32("ln2_b")[:, :, None],
        "ff_W1": f32("ff_W1"), "ff_b1T": ff_b1T,
        "ff_W2": f32("ff_W2"), "ff_b2": f32("ff_b2"),
    }
    in_maps = []
    for dcore in range(CORES):
        xs = x[dcore * nloc:(dcore + 1) * nloc]
        if npad != nloc:
            xs = np.concatenate(
                [xs, np.zeros((npad - nloc, D), np.float32)], axis=0)
        m = {"x": np.ascontiguousarray(xs), **in_edge[dcore], **shared}
        in_maps.append(m)
    return nloc, npad, ksub, in_maps


# ----------------------------------------------------------------------------
# PJRT runner (build once, reuse executable)
# ----------------------------------------------------------------------------

_CACHE = {}


def _make_runner(nc, n_cores):
    import time
    import jax
    import jax.numpy as jnp
    from jax.sharding import Mesh, PartitionSpec, NamedSharding
    from jax.experimental.shard_map import shard_map
    from concourse.bass2jax import _bass_exec_p, partition_id_tensor

    in_names, out_names, out_avals = [], [], []
    pname = nc.partition_id_tensor.name if nc.partition_id_tensor else None
    for alloc in nc.m.functions[0].allocations:
        if not isinstance(alloc, mybir.MemoryLocationSet):
            continue
        nm = alloc.memorylocations[0].name
        if alloc.kind == "ExternalInput" and nm != pname:
            in_names.append(nm)
        elif alloc.kind == "ExternalOutput":
            out_names.append(nm)
            out_avals.append(jax.core.ShapedArray(
                tuple(alloc.tensor_shape), mybir.dt.np(alloc.dtype)))
    n_params, n_outs = len(in_names), len(out_names)
    all_names = in_names + out_names + ([pname] if pname else [])
    donate = tuple(range(n_params, n_params + n_outs))

    def _body(*args):
        operands = list(args)
        if pname:
            operands.append(partition_id_tensor())
        return tuple(_bass_exec_p.bind(
            *operands, out_avals=tuple(out_avals), in_names=tuple(all_names),
            out_names=tuple(out_names), lowering_input_output_aliases=(),
            sim_require_finite=False, sim_require_nnan=False, nc=nc))

    devices = jax.devices()[:n_cores]
    mesh = Mesh(np.asarray(devices), ("core",))
    sharded = jax.jit(
        shard_map(_body, mesh=mesh,
                  in_specs=(PartitionSpec("core"),) * (n_params + n_outs),
                  out_specs=(PartitionSpec("core"),) * n_outs,
                  check_rep=False),
        donate_argnums=donate, keep_unused=True)
    shard = NamedSharding(mesh, PartitionSpec("core"))
    zero_shapes = [(n_cores * a.shape[0], *a.shape[1:]) for a in out_avals]
    zero_dtypes = [a.dtype for a in out_avals]
    make_zeros = jax.jit(
        lambda: tuple(jnp.zeros(s, d) for s, d in zip(zero_shapes, zero_dtypes)),
        out_shardings=tuple(shard for _ in out_avals))

    def run(in_maps, n_timing_iters=0, return_caller=False):
        concat_in = [
            jax.device_put(np.concatenate(
                [np.ascontiguousarray(m[nm]) for m in in_maps], axis=0), shard)
            for nm in in_names
        ]

        def call():
            zeros = make_zeros()
            jax.block_until_ready(zeros)
            t0 = time.perf_counter()
            out = sharded(*concat_in, *zeros)
            jax.block_until_ready(out)
            return out, time.perf_counter() - t0

        out_arrs = None
        for attempt in range(3):
            try:
                out_arrs, _ = call()
                break
            except Exception:
                if attempt == 2:
                    raise
                time.sleep(10.0)
        best = None
        for _ in range(n_timing_iters):
            out_arrs, dt = call()
            best = dt if best is None else min(best, dt)
        results = [
            {nm: np.asarray(out_arrs[i]).reshape(n_cores, *out_avals[i].shape)[c]
             for i, nm in enumerate(out_names)}
            for c in range(n_cores)
        ]
        if return_caller:
            return results, (lambda: call()[1] * 1e9)
        return results, (None if best is None else best * 1e9)

    return run


def run_kernel(inputs, n_timing_iters=0):
    nloc, npad, ksub, in_maps = make_in_maps(inputs)
    key = (npad, ksub)
    if key not in _CACHE:
        nc = build_nc(npad, ksub)
        _CACHE[key] = _make_runner(nc, CORES)
    results, best_ns = _CACHE[key](in_maps, n_timing_iters=n_timing_iters)
    out = np.concatenate([r["x_out"][:nloc] for r in results], axis=0)
    return out, best_ns


def kernel(**inputs):
    out, _ = run_kernel(inputs)
    return out

